# revision 1
# baseline (speedup 1.0000x reference)
"""CTRGC Trainium2 kernel.

Reference computation (per sample n):
  g     = Wg @ x[n] + bg                      [64, T=128, V=25]
  xm    = mean_t x[n]                         [64, 25]
  theta = Wth @ xm + bth ;  phi = Wph @ xm + bph        [16, 25]
  rel[i,a,b]  = tanh(theta[i,a] - phi[i,b])   [16, 25, 25]
  rel2        = Wr @ rel + br                 [64, 25, 25]
  A_dyn[c,a,b] = (A+PA)[a,b] + alpha*rel2[c,a,b]
  out[c,t,u]  = sum_v g[c,t,v] * A_dyn[c,u,v]

Sharding: data-parallel over N=128 samples across 8 cores (16 each).
On-core dataflow processes samples in pairs (2x64 channels = 128 partitions):
  MM(g, v-major) -> PE-transpose x2 (g -> gT with (ch,v) on partitions)
  -> row-tiled per-channel matmuls contracting v -> out [t, (c,u)] -> DMA.
"""

import os
import sys

import numpy as np

sys.path.insert(0, "/opt/trn_rl_repo")

import concourse.bass as bass  # noqa: E402
import concourse.tile as tile  # noqa: E402
from concourse import bacc  # noqa: E402
from concourse import mybir  # noqa: E402
from concourse.bass_utils import run_bass_kernel_spmd  # noqa: E402

F32 = mybir.dt.float32
F32R = mybir.dt.float32r
BF16 = mybir.dt.bfloat16

N, C_IN, C_OUT, C_INT, T, V = 128, 64, 64, 16, 128, 25
NCORES = 8
NSH = N // NCORES          # samples per core (16)
NPAIR = NSH // 2           # pairs per core (8)
TV = T * V                 # 3200
CU = C_OUT * V             # 1600

# bf16 pipeline for the transpose/einsum stages (g, A_dyn); fp32 accumulate.
USE_BF16 = True
GDT = BF16 if USE_BF16 else F32
STAGE = int(os.environ.get("CTRGC_STAGE", "6"))

_cache = {}


def _chunks(total, step):
    out = []
    s = 0
    while s < total:
        out.append((s, min(step, total - s)))
        s += step
    return out



def _copy(nc, eng, out_ap, in_ap):
    if eng is nc.vector:
        nc.vector.tensor_copy(out_ap, in_ap)
    else:
        nc.scalar.copy(out_ap, in_ap)


def _bias_copy(nc, eng, out_ap, in_ap, bias_ap):
    if eng is nc.vector:
        nc.vector.tensor_scalar(out=out_ap, in0=in_ap, scalar1=bias_ap,
                                scalar2=None, op0=mybir.AluOpType.add)
    else:
        nc.scalar.activation(out_ap, in_ap,
                             mybir.ActivationFunctionType.Identity,
                             bias=bias_ap)


def _build_nc():
    nc = bacc.Bacc("TRN2", target_bir_lowering=False, debug=False)

    xs_d = nc.dram_tensor("xs", [NSH, C_IN, TV], F32, kind="ExternalInput")
    ys_d = nc.dram_tensor("ys", [NSH, C_OUT, T, V], F32, kind="ExternalOutput")

    # host-prepared params, packed into 2 buffers (fewer DMA sem lanes)
    ca_d = nc.dram_tensor("constsA", [128, 629], F32, kind="ExternalInput")
    cb_d = nc.dram_tensor("constsB", [128, 448], BF16, kind="ExternalInput")

    with tile.TileContext(nc) as tc:
        _body(nc, tc, xs_d, ys_d, ca_d, cb_d)
    nc.finalize()
    return nc


def _body(nc, tc, xs_d, ys_d, ca_d, cb_d):
    from contextlib import ExitStack
    ctx = ExitStack()
    with ctx:
        const = ctx.enter_context(tc.tile_pool(name="const", bufs=1))
        xin = ctx.enter_context(tc.tile_pool(name="xin", bufs=2))
        gvp = ctx.enter_context(tc.tile_pool(name="gv", bufs=2))
        gttp = ctx.enter_context(tc.tile_pool(name="gtt", bufs=2))
        gt4p = ctx.enter_context(tc.tile_pool(name="gt4", bufs=2))
        adp = ctx.enter_context(tc.tile_pool(name="ad", bufs=2))
        outp = ctx.enter_context(tc.tile_pool(name="outs", bufs=2))
        smallp = ctx.enter_context(tc.tile_pool(name="small", bufs=3))

        psg = ctx.enter_context(tc.tile_pool(name="psg", bufs=1, space="PSUM"))
        pst2 = ctx.enter_context(tc.tile_pool(name="pst2", bufs=1, space="PSUM"))
        pst3 = ctx.enter_context(tc.tile_pool(name="pst3", bufs=1, space="PSUM"))
        ps7 = ctx.enter_context(tc.tile_pool(name="ps7", bufs=1, space="PSUM"))
        psaux = ctx.enter_context(tc.tile_pool(name="psaux", bufs=1, space="PSUM"))
        psadt = psaux

        # constants (2 packed DMAs)
        cA = const.tile([128, 629], F32)
        nc.sync.dma_start(cA[:], ca_d[:])
        cB = const.tile([128, 448], BF16)
        nc.sync.dma_start(cB[:], cb_d[:])

        bgp = cA[:, 0:1]
        bthp = cA[0:32, 1:2]
        bphp = cA[0:32, 2:3]
        abrp = cA[:, 3:4]
        strep = cA[:, 4:629]
        wgT = cB[:, 0:128]
        wthT = cB[:, 128:160]
        wphT = cB[:, 160:192]
        wrTa = cB[0:32, 192:320]
        tident = cB[:, 320:448]

        for p in range(NPAIR):
            # ---- load x pair: [128 (2s x 64cin), 3200 (t,v)] ----
            xp = xin.tile([128, TV], BF16, tag="xp")
            nc.gpsimd.dma_start(
                xp[:], xs_d[2 * p:2 * p + 2].rearrange("n c f -> (n c) f"))

            # ---- g = blockdiag(WgT) @ x, streamed v-major ----
            # gv free layout: (v, t) = v*128 + t ; bias added in copy; cast GDT
            gv = gvp.tile([128, TV], GDT, tag="gv")
            x_vmaj = xp[:].rearrange("p (t v) -> p v t", v=V)
            for ci, (v0, vn) in enumerate(_chunks(V, 4)):
                cn = vn * T
                gps = psg.tile([128, 512], F32, tag="gps")
                nc.tensor.matmul(gps[:, 0:cn], wgT,
                                 x_vmaj[:, v0:v0 + vn, :],
                                 start=True, stop=True)
                eng = nc.vector if ci % 2 == 0 else nc.scalar
                _bias_copy(nc, eng, gv[:, v0 * T:v0 * T + cn], gps[:, 0:cn],
                           bgp)

            # ---- x_sum over t -> theta/phi -> rel -> rel2 -> A_dynT ----
            if STAGE < 2:
                out_sb = outp.tile([128, 2 * CU], F32, tag="outsb")
                nc.gpsimd.memset(out_sb[:], 0.0)
                for s in range(2):
                    nc.sync.dma_start(
                        ys_d[2 * p + s].rearrange("c t u -> t c u"),
                        out_sb[:, s * CU:(s + 1) * CU].rearrange(
                            "p (c u) -> p c u", u=V))
                continue
            xsum = smallp.tile([128, V], F32, tag="xsum")
            nc.vector.tensor_reduce(
                out=xsum[:], in_=xp[:].rearrange("p (t v) -> p v t", v=V),
                axis=mybir.AxisListType.X, op=mybir.AluOpType.add)
            xsumb = smallp.tile([128, V], BF16, tag="xsumb")
            nc.vector.tensor_copy(xsumb[:], xsum[:])

            thps = psaux.tile([128, 512], F32, tag="auxps")
            nc.tensor.matmul(thps[0:32, 0:V], wthT, xsumb[:], start=True, stop=True)
            th = smallp.tile([32, V], F32, tag="th")
            nc.vector.tensor_scalar(
                out=th[:], in0=thps[0:32, 0:V], scalar1=bthp, scalar2=None,
                op0=mybir.AluOpType.add)

            phps = psaux.tile([128, 512], F32, tag="auxps")
            nc.tensor.matmul(phps[0:32, 0:V], wphT, xsumb[:], start=True, stop=True)
            ph = smallp.tile([32, V], F32, tag="ph")
            nc.vector.tensor_scalar(
                out=ph[:], in0=phps[0:32, 0:V], scalar1=bphp, scalar2=None,
                op0=mybir.AluOpType.add)

            # rel[i, (v,u)] = tanh(th[i,u] - ph[i,v])   (free = v*25+u)
            reld = smallp.tile([32, V * V], F32, tag="reld")
            th_b = th[:].rearrange("p (o u) -> p o u", o=1).broadcast_to(
                [32, V, V])
            ph_b = ph[:].broadcast_to([32, V, V])
            nc.vector.tensor_tensor(
                out=reld[:].rearrange("p (v u) -> p v u", v=V),
                in0=th_b, in1=ph_b, op=mybir.AluOpType.subtract)
            relt = smallp.tile([32, V * V], BF16, tag="relt")
            nc.scalar.activation(
                relt[:], reld[:], mybir.ActivationFunctionType.Tanh)

            # rel2 = (alpha*Wr) @ rel + alpha*br ; + S^T rep -> A_dynT flat
            advu = adp.tile([128, V * V], GDT, tag="advu")
            for c0, cn in _chunks(V * V, 512):
                r2ps = psaux.tile([128, 512], F32, tag="auxps")
                nc.tensor.matmul(r2ps[:, 0:cn], wrTa, relt[:, c0:c0 + cn],
                                 start=True, stop=True)
                tmp = smallp.tile([128, 512], F32, tag="adtmp")
                nc.vector.tensor_scalar(
                    out=tmp[:, 0:cn], in0=r2ps[:, 0:cn], scalar1=abrp,
                    scalar2=None, op0=mybir.AluOpType.add)
                nc.vector.tensor_tensor(
                    out=advu[:, c0:c0 + cn], in0=tmp[:, 0:cn],
                    in1=strep[:, c0:c0 + cn], op=mybir.AluOpType.add)

            if STAGE < 3:
                out_sb = outp.tile([128, 2 * CU], F32, tag="outsb")
                nc.gpsimd.memset(out_sb[:], 0.0)
                for s in range(2):
                    nc.sync.dma_start(
                        ys_d[2 * p + s].rearrange("c t u -> t c u"),
                        out_sb[:, s * CU:(s + 1) * CU].rearrange(
                            "p (c u) -> p c u", u=V))
                continue
            # ---- A_dynT unflatten: per-u transposes -> adtt [25v, (u, c')] ----
            adtt = adp.tile([128, V * 128], GDT, tag="adtt")
            ad_vu = advu[:].rearrange("p (v u) -> p u v", u=V)
            for b0, bn in _chunks(V, 4):
                atps = psadt.tile([32, 512], GDT, tag="auxps")
                for ui in range(bn):
                    nc.tensor.transpose(
                        atps[0:V, ui * 128:ui * 128 + 128],
                        ad_vu[:, b0 + ui, :], tident)
                eng = nc.vector if (b0 // 4) % 2 == 0 else nc.scalar
                _copy(nc, eng, adtt[0:V, b0 * 128:(b0 + bn) * 128],
                      atps[0:V, 0:bn * 128])
            # replicate to partition strips 32,64,96 for row-tiled matmuls
            for q in range(1, 4):
                nc.sync.dma_start(
                    adtt[32 * q:32 * q + V, :], adtt[0:V, :])

            if STAGE < 4:
                out_sb = outp.tile([128, 2 * CU], F32, tag="outsb")
                nc.gpsimd.memset(out_sb[:], 0.0)
                for s in range(2):
                    nc.sync.dma_start(
                        ys_d[2 * p + s].rearrange("c t u -> t c u"),
                        out_sb[:, s * CU:(s + 1) * CU].rearrange(
                            "p (c u) -> p c u", u=V))
                continue
            # ---- gT stage2: transpose v-slices -> gtt [t, c'*32 + v] ----
            gtt = gttp.tile([128, 128 * 32], GDT, tag="gtt")
            # zero the v-pad slots (v in [25,32)) so stage3 reads are finite
            nc.gpsimd.memset(
                gtt[:].rearrange("p (c v) -> p c v", v=32)[:, :, V:32], 0.0)
            for b0, bn in _chunks(V, 4):
                t2ps = pst2.tile([128, 512], GDT, tag="t2ps")
                for vi in range(bn):
                    nc.tensor.transpose(
                        t2ps[:, vi * 128:vi * 128 + 128],
                        gv[:, (b0 + vi) * 128:(b0 + vi) * 128 + 128],
                        tident)
                # dst free = c'*32 + v ; src free = vi*128 + c'
                dst = gtt[:].rearrange("p (c v) -> p v c", v=32)
                eng = nc.vector if (b0 // 4) % 2 == 1 else nc.scalar
                _copy(nc, eng, dst[:, b0:b0 + bn, :],
                      t2ps[:, 0:bn * 128].rearrange("p (v c) -> p v c", v=bn))

            if STAGE < 5:
                out_sb = outp.tile([128, 2 * CU], F32, tag="outsb")
                nc.gpsimd.memset(out_sb[:], 0.0)
                for s in range(2):
                    nc.sync.dma_start(
                        ys_d[2 * p + s].rearrange("c t u -> t c u"),
                        out_sb[:, s * CU:(s + 1) * CU].rearrange(
                            "p (c u) -> p c u", u=V))
                continue
            # ---- gT stage3: transpose 128-col slabs -> gt4 [(4ch,32v), t] ----
            gt4 = gt4p.tile([128, 32 * 128], GDT, tag="gt4")
            for b0, bn in _chunks(32, 4):
                t3ps = pst3.tile([128, 512], GDT, tag="t3ps")
                for si in range(bn):
                    g4i = b0 + si
                    nc.tensor.transpose(
                        t3ps[:, si * 128:si * 128 + 128],
                        gtt[:, g4i * 128:g4i * 128 + 128], tident)
                eng = nc.vector if (b0 // 4) % 2 == 0 else nc.scalar
                _copy(nc, eng, gt4[:, b0 * 128:(b0 + bn) * 128],
                      t3ps[:, 0:bn * 128])

            if STAGE < 6:
                out_sb = outp.tile([128, 2 * CU], F32, tag="outsb")
                nc.gpsimd.memset(out_sb[:], 0.0)
                for s in range(2):
                    nc.sync.dma_start(
                        ys_d[2 * p + s].rearrange("c t u -> t c u"),
                        out_sb[:, s * CU:(s + 1) * CU].rearrange(
                            "p (c u) -> p c u", u=V))
                continue
            # ---- step 7: per-channel row-tiled matmuls, out [t, (s,c,u)] ----
            out_sb = outp.tile([128, 2 * CU], F32, tag="outsb")
            for b0, bn in _chunks(32, 5):
                opsq = []
                for q in range(4):
                    opsq_t = ps7.tile([128, 125], F32, tag=f"ops{q}",
                                      name=f"ops{q}_{p}_{b0}")
                    opsq.append(opsq_t)
                for li in range(bn):
                    g4i = b0 + li
                    for q in range(4):
                        lhsT = gt4[32 * q:32 * q + V,
                                   g4i * 128:g4i * 128 + 128]
                        rhs = adtt[32 * q:32 * q + V, :].rearrange(
                            "p (u c) -> p c u", c=128)[:, 4 * g4i + q, :]
                        nc.tensor.matmul(
                            opsq[q][:, li * V:(li + 1) * V],
                            lhsT, rhs, start=True, stop=True,
                            tile_position=(32 * q, 0))
                dst = out_sb[:, b0 * 100:(b0 + bn) * 100].rearrange(
                    "p (g q u) -> p q g u", q=4, u=V)
                for q in range(4):
                    eng = nc.vector if q % 2 == 0 else nc.scalar
                    _copy(nc, eng, dst[:, q, :, :],
                          opsq[q][:, 0:bn * V].rearrange(
                              "p (g u) -> p g u", u=V))
                del opsq

            # ---- store: per sample, dst [c,t,u] from src [t,(c,u)] ----
            for s in range(2):
                nc.sync.dma_start(
                    ys_d[2 * p + s].rearrange("c t u -> t c u"),
                    out_sb[:, s * CU:(s + 1) * CU].rearrange(
                        "p (c u) -> p c u", u=V))


def _host_params(A, PA, alpha, Wg, bg, Wth, bth, Wph, bph, Wr, br):
    f = np.float32
    al = np.float32(alpha[0])
    wgT = np.zeros((128, 128), f)
    wgT[:64, :64] = Wg.T
    wgT[64:, 64:] = Wg.T
    # x_mean: fold 1/T into Wth/Wph lhsT
    wthT = np.zeros((128, 32), f)
    wthT[:64, :16] = Wth.T / T
    wthT[64:, 16:] = Wth.T / T
    wphT = np.zeros((128, 32), f)
    wphT[:64, :16] = Wph.T / T
    wphT[64:, 16:] = Wph.T / T
    wrTa = np.zeros((32, 128), f)
    wrTa[:16, :64] = al * Wr.T
    wrTa[16:, 64:] = al * Wr.T
    bgp = np.concatenate([bg, bg]).astype(f).reshape(128, 1)
    bthp = np.concatenate([bth, bth]).astype(f).reshape(32, 1)
    bphp = np.concatenate([bph, bph]).astype(f).reshape(32, 1)
    abrp = (al * np.concatenate([br, br])).astype(f).reshape(128, 1)
    S = (A + PA).astype(f)
    strep = np.tile(S.T.reshape(1, -1), (128, 1)).astype(f)  # [v,u] flat
    cA = np.zeros((128, 629), f)
    cA[:, 0:1] = bgp
    cA[0:32, 1:2] = bthp
    cA[0:32, 2:3] = bphp
    cA[:, 3:4] = abrp
    cA[:, 4:629] = strep
    bf16 = __import__("ml_dtypes").bfloat16
    cB = np.zeros((128, 448), f)
    cB[:, 0:128] = wgT
    cB[:, 128:160] = wthT
    cB[:, 160:192] = wphT
    cB[0:32, 192:320] = wrTa
    cB[:, 320:448] = np.eye(128, dtype=f)
    return dict(constsA=cA, constsB=cB.astype(bf16))


def kernel(**inputs):
    x = np.asarray(inputs["x"], np.float32)
    params = _host_params(
        np.asarray(inputs["A"], np.float32), np.asarray(inputs["PA"], np.float32),
        np.asarray(inputs["alpha"], np.float32), np.asarray(inputs["Wg"], np.float32),
        np.asarray(inputs["bg"], np.float32), np.asarray(inputs["Wth"], np.float32),
        np.asarray(inputs["bth"], np.float32), np.asarray(inputs["Wph"], np.float32),
        np.asarray(inputs["bph"], np.float32), np.asarray(inputs["Wr"], np.float32),
        np.asarray(inputs["br"], np.float32))

    if "nc" not in _cache:
        _cache["nc"] = _build_nc()
    nc = _cache["nc"]

    in_maps = []
    for i in range(NCORES):
        shard = x[i * NSH:(i + 1) * NSH].reshape(NSH, C_IN, TV).copy()
        m = {"xs": shard}
        m.update(params)
        in_maps.append(m)

    res = run_bass_kernel_spmd(nc, in_maps, list(range(NCORES)),
                               **_cache.get("run_kwargs", {}))
    out = np.concatenate([res.results[i]["ys"] for i in range(NCORES)], axis=0)
    _cache["last_results"] = res
    return out.astype(np.float32)


if __name__ == "__main__":
    nc = _build_nc()
    print("build ok:", len(nc.instructions) if hasattr(nc, "instructions") else "nc")



# revision 6
# speedup vs baseline: 1.7332x; 1.7332x over previous
"""CTRGC Trainium2 kernel.

Reference computation (per sample n):
  g     = Wg @ x[n] + bg                      [64, T=128, V=25]
  xm    = mean_t x[n]                         [64, 25]
  theta = Wth @ xm + bth ;  phi = Wph @ xm + bph        [16, 25]
  rel[i,a,b]  = tanh(theta[i,a] - phi[i,b])   [16, 25, 25]
  rel2        = Wr @ rel + br                 [64, 25, 25]
  A_dyn[c,a,b] = (A+PA)[a,b] + alpha*rel2[c,a,b]
  out[c,t,u]  = sum_v g[c,t,v] * A_dyn[c,u,v]

Sharding: data-parallel over N=128 samples across 8 cores (16 each).
On-core dataflow processes samples in pairs (2x64 channels = 128 partitions):
  MM(g, v-major) -> PE-transpose x2 (g -> gT with (ch,v) on partitions)
  -> row-tiled per-channel matmuls contracting v -> out [t, (c,u)] -> DMA.
"""

import os
import sys

import numpy as np

sys.path.insert(0, "/opt/trn_rl_repo")

import concourse.bass as bass  # noqa: E402
import concourse.tile as tile  # noqa: E402
from concourse import bacc  # noqa: E402
from concourse import mybir  # noqa: E402
from concourse.bass_utils import run_bass_kernel_spmd  # noqa: E402

F32 = mybir.dt.float32
F32R = mybir.dt.float32r
BF16 = mybir.dt.bfloat16

N, C_IN, C_OUT, C_INT, T, V = 128, 64, 64, 16, 128, 25
NCORES = 8
NSH = N // NCORES          # samples per core (16)
NPAIR = NSH // 2           # pairs per core (8)
TV = T * V                 # 3200
CU = C_OUT * V             # 1600

# bf16 pipeline for the transpose/einsum stages (g, A_dyn); fp32 accumulate.
USE_BF16 = True
GDT = BF16 if USE_BF16 else F32
STAGE = int(os.environ.get("CTRGC_STAGE", "6"))

_cache = {}


def _chunks(total, step):
    out = []
    s = 0
    while s < total:
        out.append((s, min(step, total - s)))
        s += step
    return out



def _copy(nc, eng, out_ap, in_ap):
    if eng is nc.vector:
        nc.vector.tensor_copy(out_ap, in_ap)
    else:
        nc.scalar.copy(out_ap, in_ap)


def _bias_copy(nc, eng, out_ap, in_ap, bias_ap):
    if eng is nc.vector:
        nc.vector.tensor_scalar(out=out_ap, in0=in_ap, scalar1=bias_ap,
                                scalar2=None, op0=mybir.AluOpType.add)
    else:
        nc.scalar.activation(out_ap, in_ap,
                             mybir.ActivationFunctionType.Identity,
                             bias=bias_ap)


def _build_nc():
    nc = bacc.Bacc("TRN2", target_bir_lowering=False, debug=False)

    xs_d = nc.dram_tensor("xs", [NSH, C_IN, TV], F32, kind="ExternalInput")
    # out stored [T, C*V] bf16 per sample; host transposes to [C, T, V] f32
    ys_d = nc.dram_tensor("ys", [NSH, T, CU], BF16, kind="ExternalOutput")

    # host-prepared params, packed into 2 buffers (fewer DMA sem lanes)
    ca_d = nc.dram_tensor("constsA", [128, 629], F32, kind="ExternalInput")
    cb_d = nc.dram_tensor("constsB", [128, 448], BF16, kind="ExternalInput")

    with tile.TileContext(nc) as tc:
        _body(nc, tc, xs_d, ys_d, ca_d, cb_d)
    nc.finalize()
    return nc


def _body(nc, tc, xs_d, ys_d, ca_d, cb_d):
    from contextlib import ExitStack
    ctx = ExitStack()
    with ctx:
        const = ctx.enter_context(tc.tile_pool(name="const", bufs=1))
        xin = ctx.enter_context(tc.tile_pool(name="xin", bufs=2))
        gvp = ctx.enter_context(tc.tile_pool(name="gv", bufs=2))
        gttp = ctx.enter_context(tc.tile_pool(name="gtt", bufs=2))
        gt4p = ctx.enter_context(tc.tile_pool(name="gt4", bufs=2))
        adp = ctx.enter_context(tc.tile_pool(name="ad", bufs=2))
        outp = ctx.enter_context(tc.tile_pool(name="outs", bufs=2))
        smallp = ctx.enter_context(tc.tile_pool(name="small", bufs=3))

        psg = ctx.enter_context(tc.tile_pool(name="psg", bufs=1, space="PSUM"))
        pst2 = ctx.enter_context(tc.tile_pool(name="pst2", bufs=1, space="PSUM"))
        pst3 = ctx.enter_context(tc.tile_pool(name="pst3", bufs=1, space="PSUM"))
        ps7 = ctx.enter_context(tc.tile_pool(name="ps7", bufs=1, space="PSUM"))
        psaux = ctx.enter_context(tc.tile_pool(name="psaux", bufs=1, space="PSUM"))
        psadt = psaux

        # constants (2 packed DMAs)
        cA = const.tile([128, 629], F32)
        nc.sync.dma_start(cA[:], ca_d[:])
        cB = const.tile([128, 448], BF16)
        nc.sync.dma_start(cB[:], cb_d[:])

        bgp = cA[:, 0:1]
        bthp = cA[0:32, 1:2]
        bphp = cA[0:32, 2:3]
        abrp = cA[:, 3:4]
        strep = cA[:, 4:629]
        wgT = cB[:, 0:128]
        wthT = cB[:, 128:160]
        wphT = cB[:, 160:192]
        wrTa = cB[0:32, 192:320]
        tident = cB[:, 320:448]

        for p in range(NPAIR):
            # ---- load x pair: [128 (2s x 64cin), 3200 (t,v)] ----
            xp = xin.tile([128, TV], BF16, tag="xp")
            nc.gpsimd.dma_start(
                xp[:], xs_d[2 * p:2 * p + 2].rearrange("n c f -> (n c) f"))

            # ---- g = blockdiag(WgT) @ x, streamed v-major ----
            # gv free layout: (v, t) = v*128 + t ; bias added in copy; cast GDT
            gv = gvp.tile([128, TV], GDT, tag="gv")
            x_vmaj = xp[:].rearrange("p (t v) -> p v t", v=V)
            for ci, (v0, vn) in enumerate(_chunks(V, 4)):
                cn = vn * T
                gps = psg.tile([128, 512], F32, tag="gps")
                nc.tensor.matmul(gps[:, 0:cn], wgT,
                                 x_vmaj[:, v0:v0 + vn, :],
                                 start=True, stop=True)
                eng = nc.vector if ci % 2 == 0 else nc.scalar
                _bias_copy(nc, eng, gv[:, v0 * T:v0 * T + cn], gps[:, 0:cn],
                           bgp)

            # ---- x_sum over t -> theta/phi -> rel -> rel2 -> A_dynT ----
            if STAGE < 2:
                out_sb = outp.tile([128, 2 * CU], BF16, tag="outsb")
                nc.gpsimd.memset(out_sb[:], 0.0)
                for s in range(2):
                    nc.sync.dma_start(ys_d[2 * p + s],
                                      out_sb[:, s * CU:(s + 1) * CU])
                continue
            xsum = smallp.tile([128, V], F32, tag="xsum")
            nc.vector.tensor_reduce(
                out=xsum[:], in_=xp[:].rearrange("p (t v) -> p v t", v=V),
                axis=mybir.AxisListType.X, op=mybir.AluOpType.add)
            xsumb = smallp.tile([128, V], BF16, tag="xsumb")
            nc.vector.tensor_copy(xsumb[:], xsum[:])

            thps = psaux.tile([128, 512], F32, tag="auxps")
            nc.tensor.matmul(thps[0:32, 0:V], wthT, xsumb[:], start=True, stop=True)
            th = smallp.tile([32, V], F32, tag="th")
            nc.vector.tensor_scalar(
                out=th[:], in0=thps[0:32, 0:V], scalar1=bthp, scalar2=None,
                op0=mybir.AluOpType.add)

            phps = psaux.tile([128, 512], F32, tag="auxps")
            nc.tensor.matmul(phps[0:32, 0:V], wphT, xsumb[:], start=True, stop=True)
            ph = smallp.tile([32, V], F32, tag="ph")
            nc.vector.tensor_scalar(
                out=ph[:], in0=phps[0:32, 0:V], scalar1=bphp, scalar2=None,
                op0=mybir.AluOpType.add)

            # rel[i, (v,u)] = tanh(th[i,u] - ph[i,v])   (free = v*25+u)
            reld = smallp.tile([32, V * V], F32, tag="reld")
            th_b = th[:].rearrange("p (o u) -> p o u", o=1).broadcast_to(
                [32, V, V])
            ph_b = ph[:].broadcast_to([32, V, V])
            nc.vector.tensor_tensor(
                out=reld[:].rearrange("p (v u) -> p v u", v=V),
                in0=th_b, in1=ph_b, op=mybir.AluOpType.subtract)
            relt = smallp.tile([32, V * V], BF16, tag="relt")
            nc.scalar.activation(
                relt[:], reld[:], mybir.ActivationFunctionType.Tanh)

            # rel2 = (alpha*Wr) @ rel + alpha*br ; + S^T rep -> A_dynT flat
            advu = adp.tile([128, V * V], GDT, tag="advu")
            for c0, cn in _chunks(V * V, 512):
                r2ps = psaux.tile([128, 512], F32, tag="auxps")
                nc.tensor.matmul(r2ps[:, 0:cn], wrTa, relt[:, c0:c0 + cn],
                                 start=True, stop=True)
                tmp = smallp.tile([128, 512], F32, tag="adtmp")
                nc.vector.tensor_scalar(
                    out=tmp[:, 0:cn], in0=r2ps[:, 0:cn], scalar1=abrp,
                    scalar2=None, op0=mybir.AluOpType.add)
                nc.vector.tensor_tensor(
                    out=advu[:, c0:c0 + cn], in0=tmp[:, 0:cn],
                    in1=strep[:, c0:c0 + cn], op=mybir.AluOpType.add)

            if STAGE < 3:
                out_sb = outp.tile([128, 2 * CU], BF16, tag="outsb")
                nc.gpsimd.memset(out_sb[:], 0.0)
                for s in range(2):
                    nc.sync.dma_start(ys_d[2 * p + s],
                                      out_sb[:, s * CU:(s + 1) * CU])
                continue
            # ---- A_dynT unflatten: per-u transposes -> adtt [25v, (u, c')] ----
            adtt = adp.tile([128, V * 128], GDT, tag="adtt")
            ad_vu = advu[:].rearrange("p (v u) -> p u v", u=V)
            for b0, bn in _chunks(V, 4):
                atps = psadt.tile([32, 512], GDT, tag="auxps")
                for ui in range(bn):
                    nc.tensor.transpose(
                        atps[0:V, ui * 128:ui * 128 + 128],
                        ad_vu[:, b0 + ui, :], tident)
                eng = nc.vector if (b0 // 4) % 2 == 0 else nc.scalar
                _copy(nc, eng, adtt[0:V, b0 * 128:(b0 + bn) * 128],
                      atps[0:V, 0:bn * 128])
            # replicate to partition strips 32,64,96 for row-tiled matmuls
            for q in range(1, 4):
                nc.sync.dma_start(
                    adtt[32 * q:32 * q + V, :], adtt[0:V, :])

            if STAGE < 4:
                out_sb = outp.tile([128, 2 * CU], BF16, tag="outsb")
                nc.gpsimd.memset(out_sb[:], 0.0)
                for s in range(2):
                    nc.sync.dma_start(ys_d[2 * p + s],
                                      out_sb[:, s * CU:(s + 1) * CU])
                continue
            # ---- gT stage2: transpose v-slices -> gtt [t, c'*32 + v] ----
            gtt = gttp.tile([128, 128 * 32], GDT, tag="gtt")
            # zero the v-pad slots (v in [25,32)) so stage3 reads are finite
            nc.gpsimd.memset(
                gtt[:].rearrange("p (c v) -> p c v", v=32)[:, :, V:32], 0.0)
            for b0, bn in _chunks(V, 4):
                t2ps = pst2.tile([128, 512], GDT, tag="t2ps")
                for vi in range(bn):
                    nc.tensor.transpose(
                        t2ps[:, vi * 128:vi * 128 + 128],
                        gv[:, (b0 + vi) * 128:(b0 + vi) * 128 + 128],
                        tident)
                # dst free = c'*32 + v ; src free = vi*128 + c'
                dst = gtt[:].rearrange("p (c v) -> p v c", v=32)
                eng = nc.vector if (b0 // 4) % 2 == 1 else nc.scalar
                _copy(nc, eng, dst[:, b0:b0 + bn, :],
                      t2ps[:, 0:bn * 128].rearrange("p (v c) -> p v c", v=bn))

            if STAGE < 5:
                out_sb = outp.tile([128, 2 * CU], BF16, tag="outsb")
                nc.gpsimd.memset(out_sb[:], 0.0)
                for s in range(2):
                    nc.sync.dma_start(ys_d[2 * p + s],
                                      out_sb[:, s * CU:(s + 1) * CU])
                continue
            # ---- gT stage3: transpose 128-col slabs -> gt4 [(4ch,32v), t] ----
            gt4 = gt4p.tile([128, 32 * 128], GDT, tag="gt4")
            for b0, bn in _chunks(32, 4):
                t3ps = pst3.tile([128, 512], GDT, tag="t3ps")
                for si in range(bn):
                    g4i = b0 + si
                    nc.tensor.transpose(
                        t3ps[:, si * 128:si * 128 + 128],
                        gtt[:, g4i * 128:g4i * 128 + 128], tident)
                eng = nc.vector if (b0 // 4) % 2 == 0 else nc.scalar
                _copy(nc, eng, gt4[:, b0 * 128:(b0 + bn) * 128],
                      t3ps[:, 0:bn * 128])

            if STAGE < 6:
                out_sb = outp.tile([128, 2 * CU], BF16, tag="outsb")
                nc.gpsimd.memset(out_sb[:], 0.0)
                for s in range(2):
                    nc.sync.dma_start(ys_d[2 * p + s],
                                      out_sb[:, s * CU:(s + 1) * CU])
                continue
            # ---- step 7: per-channel row-tiled matmuls, out [t, (s,c,u)] ----
            out_sb = outp.tile([128, 2 * CU], BF16, tag="outsb")
            for b0, bn in _chunks(32, 5):
                opsq = []
                for q in range(4):
                    opsq_t = ps7.tile([128, 125], F32, tag=f"ops{q}",
                                      name=f"ops{q}_{p}_{b0}")
                    opsq.append(opsq_t)
                for li in range(bn):
                    g4i = b0 + li
                    for q in range(4):
                        lhsT = gt4[32 * q:32 * q + V,
                                   g4i * 128:g4i * 128 + 128]
                        rhs = adtt[32 * q:32 * q + V, :].rearrange(
                            "p (u c) -> p c u", c=128)[:, 4 * g4i + q, :]
                        nc.tensor.matmul(
                            opsq[q][:, li * V:(li + 1) * V],
                            lhsT, rhs, start=True, stop=True,
                            tile_position=(32 * q, 0))
                dst = out_sb[:, b0 * 100:(b0 + bn) * 100].rearrange(
                    "p (g q u) -> p q g u", q=4, u=V)
                for q in range(4):
                    eng = nc.vector if q % 2 == 0 else nc.scalar
                    _copy(nc, eng, dst[:, q, :, :],
                          opsq[q][:, 0:bn * V].rearrange(
                              "p (g u) -> p g u", u=V))
                del opsq

            # ---- store: per sample, contiguous [t, (c,u)] bf16 rows ----
            for s in range(2):
                nc.sync.dma_start(ys_d[2 * p + s],
                                  out_sb[:, s * CU:(s + 1) * CU])


def _host_params(A, PA, alpha, Wg, bg, Wth, bth, Wph, bph, Wr, br):
    f = np.float32
    al = np.float32(alpha[0])
    wgT = np.zeros((128, 128), f)
    wgT[:64, :64] = Wg.T
    wgT[64:, 64:] = Wg.T
    # x_mean: fold 1/T into Wth/Wph lhsT
    wthT = np.zeros((128, 32), f)
    wthT[:64, :16] = Wth.T / T
    wthT[64:, 16:] = Wth.T / T
    wphT = np.zeros((128, 32), f)
    wphT[:64, :16] = Wph.T / T
    wphT[64:, 16:] = Wph.T / T
    wrTa = np.zeros((32, 128), f)
    wrTa[:16, :64] = al * Wr.T
    wrTa[16:, 64:] = al * Wr.T
    bgp = np.concatenate([bg, bg]).astype(f).reshape(128, 1)
    bthp = np.concatenate([bth, bth]).astype(f).reshape(32, 1)
    bphp = np.concatenate([bph, bph]).astype(f).reshape(32, 1)
    abrp = (al * np.concatenate([br, br])).astype(f).reshape(128, 1)
    S = (A + PA).astype(f)
    strep = np.tile(S.T.reshape(1, -1), (128, 1)).astype(f)  # [v,u] flat
    cA = np.zeros((128, 629), f)
    cA[:, 0:1] = bgp
    cA[0:32, 1:2] = bthp
    cA[0:32, 2:3] = bphp
    cA[:, 3:4] = abrp
    cA[:, 4:629] = strep
    bf16 = __import__("ml_dtypes").bfloat16
    cB = np.zeros((128, 448), f)
    cB[:, 0:128] = wgT
    cB[:, 128:160] = wthT
    cB[:, 160:192] = wphT
    cB[0:32, 192:320] = wrTa
    cB[:, 320:448] = np.eye(128, dtype=f)
    return dict(constsA=cA, constsB=cB.astype(bf16))


def kernel(**inputs):
    x = np.asarray(inputs["x"], np.float32)
    params = _host_params(
        np.asarray(inputs["A"], np.float32), np.asarray(inputs["PA"], np.float32),
        np.asarray(inputs["alpha"], np.float32), np.asarray(inputs["Wg"], np.float32),
        np.asarray(inputs["bg"], np.float32), np.asarray(inputs["Wth"], np.float32),
        np.asarray(inputs["bth"], np.float32), np.asarray(inputs["Wph"], np.float32),
        np.asarray(inputs["bph"], np.float32), np.asarray(inputs["Wr"], np.float32),
        np.asarray(inputs["br"], np.float32))

    if "nc" not in _cache:
        _cache["nc"] = _build_nc()
    nc = _cache["nc"]

    in_maps = []
    for i in range(NCORES):
        shard = x[i * NSH:(i + 1) * NSH].reshape(NSH, C_IN, TV).copy()
        m = {"xs": shard}
        m.update(params)
        in_maps.append(m)

    res = run_bass_kernel_spmd(nc, in_maps, list(range(NCORES)),
                               **_cache.get("run_kwargs", {}))
    # device emits [NSH, T, C*V] bf16; unshard + fix layout to [N, C, T, V]
    out = np.concatenate([np.asarray(res.results[i]["ys"]) for i in range(NCORES)],
                         axis=0)
    out = out.reshape(N, T, C_OUT, V).transpose(0, 2, 1, 3)
    _cache["last_results"] = res
    return np.ascontiguousarray(out, dtype=np.float32)


if __name__ == "__main__":
    nc = _build_nc()
    print("build ok:", len(nc.instructions) if hasattr(nc, "instructions") else "nc")



# revision 28
# speedup vs baseline: 1.8988x; 1.0955x over previous
"""CTRGC Trainium2 kernel (v2).

Reference computation (per sample n):
  g     = Wg @ x[n] + bg                      [64, T=128, V=25]
  xm    = mean_t x[n]                         [64, 25]
  theta = Wth @ xm + bth ;  phi = Wph @ xm + bph        [16, 25]
  rel[i,a,b]  = tanh(theta[i,a] - phi[i,b])   [16, 25, 25]
  rel2        = Wr @ rel + br                 [64, 25, 25]
  A_dyn[c,a,b] = (A+PA)[a,b] + alpha*rel2[c,a,b]
  out[c,t,u]  = sum_v g[c,t,v] * A_dyn[c,u,v]

Sharding: data-parallel over N=128 samples across 8 cores (16 each),
processed on-core in pairs (2x64 channels = 128 partitions).

v2 dataflow (per pair):
  x uploaded v-major [.., (v,t)].
  g^T produced directly by matmul with x-slices stationary:
    out[t, (s,c)] per v -> gtt [t, (c',v32)]  (no PE transposes)
  one XBAR dma transpose: gtt -> gt4 [(q,v32), (grp, t)]
  A_dyn chain in u-major layout with a bias slot at v=25:
    advu [c, (u, v26)], col v=25 = bg[c]*sum_v A_dyn[c,u,v]
  step7: 128 row-tiled matmuls contract k=26 (25 v + ones row carries
    the g-bias term): out [t, (s,c,u)] -> bf16 -> DRAM [T, C*V].
  Host unshard: concat + transpose to [N, C, T, V] f32.
"""

import os
import sys

import numpy as np

sys.path.insert(0, "/opt/trn_rl_repo")

import concourse.bass as bass  # noqa: E402
import concourse.tile as tile  # noqa: E402
from concourse import bacc  # noqa: E402
from concourse import mybir  # noqa: E402
from concourse.bass_utils import run_bass_kernel_spmd  # noqa: E402

F32 = mybir.dt.float32
BF16 = mybir.dt.bfloat16

USE_POOL = os.environ.get("CTRGC_POOL", "1") == "1"
USE_XBAR = os.environ.get("CTRGC_XBAR", "1") == "1"

N, C_IN, C_OUT, C_INT, T, V = 128, 64, 64, 16, 128, 25
NCORES = 8
NSH = N // NCORES          # samples per core (16)
NPAIR = NSH // 2           # pairs per core (8)
TV = T * V                 # 3200
CU = C_OUT * V             # 1600
V1 = V + 1                 # 26: v plus the bias slot

_cache = {}


def _build_nc():
    nc = bacc.Bacc("TRN2", target_bir_lowering=False, debug=False)

    # x pre-permuted on host to v-major: [NSH, C_IN, (v,t)]
    xs_d = nc.dram_tensor("xs", [NSH, C_IN, TV], F32, kind="ExternalInput")
    # out stored [T, C*V] bf16 per sample; host transposes to [C, T, V] f32
    ys_d = nc.dram_tensor("ys", [NSH, T, CU], BF16, kind="ExternalOutput")

    ca_d = nc.dram_tensor("constsA", [128, 654], F32, kind="ExternalInput")
    cb_d = nc.dram_tensor("constsB", [128, 448], BF16, kind="ExternalInput")

    with tile.TileContext(nc) as tc:
        _body(nc, tc, xs_d, ys_d, ca_d, cb_d)
    nc.finalize()
    return nc


def _body(nc, tc, xs_d, ys_d, ca_d, cb_d):
    from contextlib import ExitStack
    ctx = ExitStack()
    with ctx:
        const = ctx.enter_context(tc.tile_pool(name="const", bufs=1))
        xin = ctx.enter_context(tc.tile_pool(name="xin", bufs=2))
        gttp = ctx.enter_context(tc.tile_pool(name="gtt", bufs=2))
        gt4p = ctx.enter_context(tc.tile_pool(name="gt4", bufs=2))
        adp = ctx.enter_context(tc.tile_pool(name="ad", bufs=2))
        outp = ctx.enter_context(tc.tile_pool(name="outs", bufs=2))
        smallp = ctx.enter_context(tc.tile_pool(name="small", bufs=3))

        psg = ctx.enter_context(tc.tile_pool(name="psg", bufs=2, space="PSUM"))
        ps7 = ctx.enter_context(tc.tile_pool(name="ps7", bufs=1, space="PSUM"))
        psaux = ctx.enter_context(tc.tile_pool(name="psaux", bufs=1,
                                               space="PSUM"))
        psadt = ctx.enter_context(tc.tile_pool(name="psadt", bufs=1,
                                               space="PSUM"))

        cA = const.tile([128, 654], F32)
        nc.sync.dma_start(cA[:], ca_d[:])
        cB = const.tile([128, 448], BF16)
        nc.sync.dma_start(cB[:], cb_d[:])

        bgp = cA[:, 0:1]
        bthp = cA[0:32, 1:2]
        bphp = cA[0:32, 2:3]
        strepA = cA[:, 4:654]          # [c, (u,v26)] S[u,v] + alpha*br[c]
        wgT = cB[:, 0:128]
        wthT = cB[:, 128:160]
        wphT = cB[:, 160:192]
        wrTa = cB[0:32, 192:320]
        tident = cB[:, 320:448]

        # copy-engine rotation, ACT-heavy (gpsimd cannot read PSUM;
        # DVE carries the reduces)
        def _copy(i, out_ap, in_ap):
            if i % 3 == 0:
                nc.vector.tensor_copy(out_ap, in_ap)
            else:
                nc.scalar.copy(out_ap, in_ap)

        for p in range(NPAIR):
            # ---- load x pair: [128 (2s x 64cin), 3200 (v,t)] ----
            xp = xin.tile([128, TV], BF16, tag="xp")
            nc.gpsimd.dma_start(
                xp[:], xs_d[2 * p:2 * p + 2].rearrange("n c f -> (n c) f"))

            # ---- g^T direct: per v, out[t, (s,c)] ; gtt [t, (c',v32)] ----
            gtt = gttp.tile([128, 128 * 32], BF16, tag="gtt")
            gtt_v = gtt[:].rearrange("p (c v) -> p v c", v=32)
            # bias slot v=25 <- 1.0 (ones row for step7 k=26); v=26.. <- 0
            nc.gpsimd.memset(gtt_v[:, V:V + 1, :], 1.0)
            nc.gpsimd.memset(gtt_v[:, V + 1:32, :], 0.0)
            ci = 0
            for v0, vn in _chunks(V, 4):
                gps = psg.tile([128, 512], F32, tag="gps")
                for vi in range(vn):
                    v = v0 + vi
                    nc.tensor.matmul(
                        gps[:, vi * 128:vi * 128 + 128],
                        xp[:, v * T:(v + 1) * T], wgT,
                        start=True, stop=True)
                _copy(ci, gtt_v[:, v0:v0 + vn, :],
                      gps[:, 0:vn * 128].rearrange("p (v c) -> p v c", c=128))
                ci += 1

            # ---- xsum over t (v-major: unit stride) -> theta/phi ----
            xsum = smallp.tile([128, V], F32, tag="xsum")
            nc.vector.tensor_reduce(
                out=xsum[:], in_=xp[:].rearrange("p (v t) -> p v t", v=V),
                axis=mybir.AxisListType.X, op=mybir.AluOpType.add)
            xsumb = smallp.tile([128, V], BF16, tag="xsumb")
            (nc.gpsimd if USE_POOL else nc.vector).tensor_copy(
                xsumb[:], xsum[:])

            thps = psaux.tile([128, 512], F32, tag="auxps")
            nc.tensor.matmul(thps[0:32, 0:V], wthT, xsumb[:],
                             start=True, stop=True)
            th = smallp.tile([32, V], F32, tag="th")
            nc.scalar.activation(th[:], thps[0:32, 0:V],
                                 mybir.ActivationFunctionType.Identity,
                                 bias=bthp)

            phps = psaux.tile([128, 512], F32, tag="auxps")
            nc.tensor.matmul(phps[0:32, 0:V], wphT, xsumb[:],
                             start=True, stop=True)
            ph = smallp.tile([32, V], F32, tag="ph")
            nc.scalar.activation(ph[:], phps[0:32, 0:V],
                                 mybir.ActivationFunctionType.Identity,
                                 bias=bphp)

            # ---- rel[i, (u,v26)] = tanh(th[i,u] - ph[i,v]) ----
            reld = smallp.tile([32, V * V1], F32, tag="reld")
            r3 = reld[:].rearrange("p (u v) -> p u v", v=V1)
            nc.gpsimd.memset(r3[:, :, V:V1], 0.0)  # bias slot: tanh(0)=0
            th_b = th[:].rearrange("p (u o) -> p u o", o=1).broadcast_to(
                [32, V, V])
            ph_b = ph[:].rearrange("p (o v) -> p o v", o=1).broadcast_to(
                [32, V, V])
            (nc.gpsimd if USE_POOL else nc.vector).tensor_tensor(
                out=r3[:, :, 0:V], in0=th_b, in1=ph_b,
                op=mybir.AluOpType.subtract)
            relt = smallp.tile([32, V * V1], BF16, tag="relt")
            nc.scalar.activation(
                relt[:], reld[:], mybir.ActivationFunctionType.Tanh)

            # ---- rel2 -> advu [c, (u, v26)] = A_dyn[c,u,v] (bf16) ----
            # strepA already carries S[u,v] + alpha*br[c] (host-folded)
            advu = adp.tile([128, V * V1], BF16, tag="advu")
            for c0, cn in _chunks(V * V1, 512):
                r2ps = psaux.tile([128, 512], F32, tag="auxps")
                nc.tensor.matmul(r2ps[:, 0:cn], wrTa, relt[:, c0:c0 + cn],
                                 start=True, stop=True)
                nc.vector.tensor_tensor(
                    out=advu[:, c0:c0 + cn], in0=r2ps[:, 0:cn],
                    in1=strepA[:, c0:c0 + cn], op=mybir.AluOpType.add)

            # bias slot v=25: bg[c] * sum_v A_dyn[c,u,v]
            ad3 = advu[:].rearrange("p (u v) -> p u v", v=V1)
            sumA = smallp.tile([128, V], F32, tag="sumA")
            nc.vector.tensor_reduce(
                out=sumA[:], in_=ad3[:, :, 0:V],
                axis=mybir.AxisListType.X, op=mybir.AluOpType.add)
            (nc.gpsimd if USE_POOL else nc.vector).tensor_scalar(
                out=ad3[:, :, V:V1],
                in0=sumA[:].rearrange("p (u o) -> p u o", o=1),
                scalar1=bgp, scalar2=None, op0=mybir.AluOpType.mult)

            # ---- adtt [v26(+strips), (c,u)] via PE transposes ----
            adtt = adp.tile([128, 128 * V], BF16, tag="adtt")
            for u0, un in _chunks(V, 4):
                atps = psadt.tile([32, 512], BF16, tag="atps")
                for ui in range(un):
                    nc.tensor.transpose(
                        atps[0:V1, ui * 128:ui * 128 + 128],
                        advu[:, (u0 + ui) * V1:(u0 + ui + 1) * V1], tident)
                _copy(ci,
                      adtt[0:V1, :].rearrange(
                          "p (c u) -> p u c", u=V)[:, u0:u0 + un, :],
                      atps[0:V1, 0:un * 128].rearrange(
                          "p (u c) -> p u c", c=128))
                ci += 1
            for q in range(1, 4):
                nc.sync.dma_start(adtt[32 * q:32 * q + V1, :],
                                  adtt[0:V1, :])

            # ---- XBAR: gtt [t, (c',v32)] -> gt4 [(q,v32), (grp, t)] ----
            gt4 = gt4p.tile([128, 32 * 128], BF16, tag="gt4")
            if USE_XBAR:
                nc.sync.dma_start_transpose(
                    out=gt4[:].rearrange("p (g t) -> p g t", t=128),
                    in_=gtt[:])
            else:
                for b0, bn in _chunks(32, 4):
                    t3ps = psg.tile([128, 512], BF16, tag="gps")
                    for si in range(bn):
                        nc.tensor.transpose(
                            t3ps[:, si * 128:si * 128 + 128],
                            gtt[:, (b0 + si) * 128:(b0 + si) * 128 + 128],
                            tident)
                    _copy(ci, gt4[:, b0 * 128:(b0 + bn) * 128],
                          t3ps[:, 0:bn * 128])
                    ci += 1

            # ---- step7: row-tiled matmuls, k=26 (v + bias row) ----
            # one PSUM tile per PE row-band q (mixing tile_positions in a
            # single PSUM tile is not safe)
            out_sb = outp.tile([128, 2 * CU], BF16, tag="outsb")
            for b0 in range(0, 32, 16):
                p7q = [ps7.tile([128, 400], F32, tag=f"p7{q}",
                                name=f"p7{q}_{p}_{b0}") for q in range(4)]
                for gi in range(16):
                    g4i = b0 + gi
                    for q in range(4):
                        c = 4 * g4i + q
                        nc.tensor.matmul(
                            p7q[q][:, gi * V:(gi + 1) * V],
                            gt4[32 * q:32 * q + V1,
                                g4i * 128:g4i * 128 + 128],
                            adtt[32 * q:32 * q + V1, c * V:(c + 1) * V],
                            start=True, stop=True,
                            tile_position=(32 * q, 0))
                dst = out_sb[:, b0 * 100:(b0 + 16) * 100].rearrange(
                    "p (g q u) -> p q g u", q=4, u=V)
                for q in range(4):
                    _copy(ci, dst[:, q, :, :],
                          p7q[q][:, 0:400].rearrange("p (g u) -> p g u", u=V))
                    ci += 1
                del p7q

            # ---- store: per sample, contiguous [t, (c,u)] bf16 rows ----
            for s in range(2):
                nc.sync.dma_start(ys_d[2 * p + s],
                                  out_sb[:, s * CU:(s + 1) * CU])


def _chunks(total, step):
    out = []
    s = 0
    while s < total:
        out.append((s, min(step, total - s)))
        s += step
    return out


def _host_params(A, PA, alpha, Wg, bg, Wth, bth, Wph, bph, Wr, br):
    f = np.float32
    al = np.float32(alpha[0])
    wgT = np.zeros((128, 128), f)
    wgT[:64, :64] = Wg.T
    wgT[64:, 64:] = Wg.T
    # x_mean: fold 1/T into Wth/Wph lhsT
    wthT = np.zeros((128, 32), f)
    wthT[:64, :16] = Wth.T / T
    wthT[64:, 16:] = Wth.T / T
    wphT = np.zeros((128, 32), f)
    wphT[:64, :16] = Wph.T / T
    wphT[64:, 16:] = Wph.T / T
    wrTa = np.zeros((32, 128), f)
    wrTa[:16, :64] = al * Wr.T
    wrTa[16:, 64:] = al * Wr.T
    bgp = np.concatenate([bg, bg]).astype(f).reshape(128, 1)
    bthp = np.concatenate([bth, bth]).astype(f).reshape(32, 1)
    bphp = np.concatenate([bph, bph]).astype(f).reshape(32, 1)
    abrp = (al * np.concatenate([br, br])).astype(f).reshape(128, 1)
    S = (A + PA).astype(f)
    # strepA[c, u*26+v] = S[u,v] + alpha*br[c] for v<25; slot v=25 -> 0
    sU = np.zeros((V, V1), f)
    sU[:, :V] = S
    strepA = np.tile(sU.reshape(1, -1), (128, 1)).astype(f)
    mask = (np.arange(V * V1) % V1 < V).astype(f).reshape(1, -1)
    strepA = strepA + abrp @ mask
    cA = np.zeros((128, 654), f)
    cA[:, 0:1] = bgp
    cA[0:32, 1:2] = bthp
    cA[0:32, 2:3] = bphp
    cA[:, 4:654] = strepA
    bf16 = __import__("ml_dtypes").bfloat16
    cB = np.zeros((128, 448), f)
    cB[:, 0:128] = wgT
    cB[:, 128:160] = wthT
    cB[:, 160:192] = wphT
    cB[0:32, 192:320] = wrTa
    cB[:, 320:448] = np.eye(128, dtype=f)
    return dict(constsA=cA, constsB=cB.astype(bf16))


def kernel(**inputs):
    x = np.asarray(inputs["x"], np.float32)
    params = _host_params(
        np.asarray(inputs["A"], np.float32), np.asarray(inputs["PA"], np.float32),
        np.asarray(inputs["alpha"], np.float32), np.asarray(inputs["Wg"], np.float32),
        np.asarray(inputs["bg"], np.float32), np.asarray(inputs["Wth"], np.float32),
        np.asarray(inputs["bth"], np.float32), np.asarray(inputs["Wph"], np.float32),
        np.asarray(inputs["bph"], np.float32), np.asarray(inputs["Wr"], np.float32),
        np.asarray(inputs["br"], np.float32))

    if "nc" not in _cache:
        _cache["nc"] = _build_nc()
    nc = _cache["nc"]

    # upload x v-major: [NSH, C_IN, (v,t)]
    xv = np.ascontiguousarray(x.transpose(0, 1, 3, 2)).reshape(N, C_IN, TV)
    in_maps = []
    for i in range(NCORES):
        m = {"xs": xv[i * NSH:(i + 1) * NSH]}
        m.update(params)
        in_maps.append(m)

    res = run_bass_kernel_spmd(nc, in_maps, list(range(NCORES)),
                               **_cache.get("run_kwargs", {}))
    # device emits [NSH, T, C*V] bf16; unshard + fix layout to [N, C, T, V]
    out = np.concatenate([np.asarray(res.results[i]["ys"]) for i in range(NCORES)],
                         axis=0)
    out = out.reshape(N, T, C_OUT, V).transpose(0, 2, 1, 3)
    _cache["last_results"] = res
    return np.ascontiguousarray(out, dtype=np.float32)


if __name__ == "__main__":
    nc = _build_nc()
    print("build ok")
